# revision 15
# baseline (speedup 1.0000x reference)
"""ContraCLM token-level contrastive loss on 8 Trainium2 NeuronCores.

Data-parallel over the batch: core b handles sample b (B=8). The token
mask is known on the host inside kernel(), so unmasked tokens are
COMPACTED host-side to a fixed padded length NCOMP (= 896 >= n + 6.5
sigma for n ~ Binom(1536, 1/2)); pad slots reuse token 0's row and get
mask 0. The device then works on a dense [2*NCOMP, 2*NCOMP] problem --
2.9x fewer matmul/exp elements than the full 2S grid.

Per core, with N_c = NCOMP, D = 1024, T = 0.05:

  f_v = l2norm(h_v) masked; rsqrt computed as exp(-0.5*ln(ss)) so the
  ScalarE only ever needs the natural_log_exp activation-table set.
  F = [f1; f2], stored transposed as fp8 [D, 2*N_c] (x8 scale).

  sim strips: per row-block r, one PSUM group [128, N_c] (<=2 banks)
  accumulates 4 DoubleRow fp8 matmuls per 512-col strip (K=1024).
  Self-similarity diagonals are killed by ONE extra tiny matmul that
  accumulates -30000*I into the diagonal 128-block, so exp() of the
  whole group rowsum-accumulates on the ScalarE free-dim accumulator
  with no fixups. The positive-pair column (block diagonal of the B
  quadrant) stays IN the row sum (denominator = Ng + pos), and pos
  itself is extracted from the exp'd block via affine_select(diag).

  C-quadrant row sums (view-2 rows vs view-1 cols) are B-quadrant
  column sums by symmetry: ones-weight matmuls accumulate [1, 512]
  PSUM rows which are transposed back to token layout on the PE.

  Masked/pad columns contribute exp(0)=1 to every row sum: corrected
  with negK0 = 2n - 2*N_c (host-computed, folded into the Ln bias).
  per-core output = sum_tok mask*(ln(Ng+pos) - ln(pos)); the host
  divides by 2n and averages across the 8 cores (no device collective).
"""

import sys

for _p in ("/opt/trn_rl_repo", "/opt/pypackages"):
    if _p not in sys.path:
        sys.path.append(_p)

from contextlib import ExitStack

import numpy as np

import bass_rust

import concourse.bass as bass
import concourse.tile as tile
from concourse import mybir
from concourse.bass_utils import run_bass_kernel_spmd
from concourse.masks import make_identity
from concourse.vector_clock import ScopedClock

# The walrus build in this container encodes at most 2 sync waits per
# instruction (bass_rust's inst_waits_full agrees), but Tile's semaphore
# assignment can attach more. Hoist excess waits onto unfusable same-engine
# NoOps immediately before the instruction -- the engine executes its queue
# in order, so semantics are preserved.
_MAX_WAITS = 1


def _split_excess_waits(nc, ordered):
    for bb_name, insts in ordered.items():
        out = []
        changed = False
        for inst in insts:
            si = getattr(inst, "sync_info", None)
            waits = list(si.on_wait) if si is not None else []
            if len(waits) > _MAX_WAITS:
                changed = True
                extra, keep = waits[:-_MAX_WAITS], waits[-_MAX_WAITS:]
                for i in range(0, len(extra), _MAX_WAITS):
                    out.append(mybir.InstNoOp(
                        name=nc.get_next_instruction_name(),
                        sync_info=mybir.SyncInfo(
                            on_wait=extra[i:i + _MAX_WAITS], on_update=[]),
                        bass_nofuse=True,
                        engine=inst.engine,
                    ))
                si.on_wait = keep
            out.append(inst)
        if changed:
            insts[:] = out


_orig_lower_ordered_insts = tile.TileContext._lower_ordered_insts


def _patched_lower_ordered_insts(self, ordered):
    _split_excess_waits(self.nc, ordered)
    return _orig_lower_ordered_insts(self, ordered)


tile.TileContext._lower_ordered_insts = _patched_lower_ordered_insts


def _split_waits_drain_and_barrier(self, tick_clock, wait_clock):
    nc = self.nc
    probe = nc.sync.nop(nofuse=True)
    wait_clock.add_sem_waits(
        probe.ins, ScopedClock({None: tick_clock.global_clock}))
    si = probe.ins.sync_info
    waits = list(si.on_wait) if si is not None else []
    if len(waits) > _MAX_WAITS:
        si.on_wait = waits[:_MAX_WAITS]
        for i in range(_MAX_WAITS, len(waits), _MAX_WAITS):
            nxt = nc.sync.nop(nofuse=True)
            nxt.ins.sync_info = bass_rust.SyncInfo(
                on_wait=waits[i:i + _MAX_WAITS], on_update=[])
    nc.sync.drain()
    nc.all_engine_barrier()
    assert self.sems is not None
    popped = nc._tile_sem_poison_stack.pop()
    assert popped is self._sem_poison
    nc.clear_and_free_semaphores(list(self.sems.allocated().values()))
    nc.all_engine_barrier()


tile.TileContext._drain_and_barrier = _split_waits_drain_and_barrier

S, D, NCORES = 1536, 1024, 8
NCOMP = 896              # padded compacted token count (multiple of 128)
KT = D // 128            # 8 contraction k-tiles
TEMP_INV = 20.0          # 1 / 0.05
FP8_SCALE = 8.0          # f entries ~N(0, 1/32); x8 keeps them in e4m3 range
NEGBIG = -30000.0        # diag killer: exp(scale*(64*sim + NEGBIG)) == 0
F32 = mybir.dt.float32
BF16 = mybir.dt.bfloat16
FP8 = mybir.dt.float8e4
AF = mybir.ActivationFunctionType
ALU = mybir.AluOpType


def _col_strips(sc):
    """Split sc (= NCOMP, multiple of 128) into <=512-wide col strips."""
    strips = []
    c = 0
    while c < sc:
        w = min(512, sc - c)
        strips.append((c, w))
        c += w
    return strips


def _build(ncomp: int) -> bass.Bass:
    st = ncomp // 128          # token tiles per view
    nb = 2 * st                # row blocks of F
    strips = _col_strips(ncomp)
    exp_scale = TEMP_INV / (FP8_SCALE * FP8_SCALE)

    nc = bass.Bass(num_devices=NCORES)
    h1 = nc.dram_tensor("h1", [ncomp, D], F32, kind="ExternalInput")
    h2 = nc.dram_tensor("h2", [ncomp, D], F32, kind="ExternalInput")
    # aux: cols 0..st-1 = mask in [128, st] layout (token t = 128*col+row),
    # col st = negK0 = 2n - 2*ncomp (broadcast down the partition dim)
    aux = nc.dram_tensor("aux", [128, st + 1], F32, kind="ExternalInput")
    out = nc.dram_tensor("loss", [1, 1], F32, kind="ExternalOutput")

    with tile.TileContext(nc) as tc, ExitStack() as ctx:
        const_pool = ctx.enter_context(tc.tile_pool(name="const", bufs=1))
        big = ctx.enter_context(tc.tile_pool(name="big", bufs=1))
        stat = ctx.enter_context(tc.tile_pool(name="stat", bufs=1))

        # Kick off all input DMAs first, split across two trigger queues
        # (Sync and Tensor) so descriptor generation doesn't serialize.
        h1b = big.tile([128, st, D], F32)
        h2b = big.tile([128, st, D], F32)
        auxs = const_pool.tile([128, st + 1], F32)
        nc.sync.dma_start(auxs[:], aux[:])
        for t in range(st):
            nc.sync.dma_start(h1b[:, t, :], h1[t * 128:(t + 1) * 128, :])
            nc.gpsimd.dma_start(h2b[:, t, :], h2[t * 128:(t + 1) * 128, :])
        msk = auxs[:, 0:st]
        negK0 = auxs[:, st:st + 1]

        ident = const_pool.tile([128, 128], BF16)
        make_identity(nc, ident[:])
        negbigI = const_pool.tile([128, 128], BF16)
        nc.vector.tensor_scalar_mul(negbigI[:], ident[:], NEGBIG)
        ones_bf = const_pool.tile([128, 1], BF16)
        nc.gpsimd.memset(ones_bf[:], 1.0)
        ones_f = const_pool.tile([128, 1], F32)
        nc.gpsimd.memset(ones_f[:], 1.0)
        ln8_col = const_pool.tile([128, 1], F32)
        nc.gpsimd.memset(ln8_col[:], float(np.log(FP8_SCALE)))
        warm_rhs = const_pool.tile([128, 512], BF16)
        nc.gpsimd.memset(warm_rhs[:], 0.0)

        fT1 = big.tile([128, KT, ncomp], FP8)     # f1^T * 8
        fT2 = big.tile([128, KT, ncomp], FP8)
        ss = stat.tile([128, nb], F32)            # |h|^2 per token, both views
        sc12 = stat.tile([128, nb], F32)          # 8 * mask * rsqrt(ss)
        accA = stat.tile([128, st], F32)          # A-quadrant row sums
        accB = stat.tile([128, nb], F32)          # B+D quadrant row sums
        pose = stat.tile([128, st], F32)          # exp(pos_sim/T) extracted
        cacc = stat.tile([128, st], F32)          # B col sums (C row sums)

        # ---- sim strips: 3 half-rows of row passes ----
        def row_pass(mmp, esp, r, lhsT, rhsT, dk_block, acc_col, want_cacc,
                     want_pos, cacc_ps, first_cacc, last_cacc):
            rT = r % st
            ps = mmp.tile([128, 1024 * ((ncomp + 1023) // 1024)], F32,
                          tag="ps", name=f"ps_{r}_{acc_col[1]}")
            for g in range(KT // 2):
                for (c0, w) in strips:
                    nc.tensor.matmul(
                        ps[:, c0:c0 + w],
                        lhsT[:, 2 * g:2 * g + 2, rT * 128:(rT + 1) * 128],
                        rhsT[:, 2 * g:2 * g + 2, c0:c0 + w],
                        perf_mode=mybir.MatmulPerfMode.DoubleRow,
                        start=(g == 0),
                        stop=(g == KT // 2 - 1 and dk_block is None))
            if dk_block is not None:
                b0 = dk_block * 128
                nc.tensor.matmul(ps[:, b0:b0 + 128], negbigI[:], ident[:],
                                 start=False, stop=True, skip_group_check=True)
            es = esp.tile([128, ncomp], BF16, tag="es", name=f"es_{r}_{acc_col[1]}")
            nc.scalar.activation(es[:], ps[:, 0:ncomp], AF.Exp,
                                 scale=exp_scale,
                                 accum_out=acc_col[0][:, acc_col[1]:acc_col[1] + 1])
            if want_cacc:
                for ci, (c0, w) in enumerate(strips):
                    nc.tensor.matmul(
                        cacc_ps[32 * ci:32 * ci + 1, 0:w],
                        ones_bf[:], es[:, c0:c0 + w],
                        start=first_cacc, stop=last_cacc,
                        skip_group_check=True)
            if want_pos:
                psel = esp.tile([128, 128], BF16, tag="psel", name=f"psel_{r}")
                blk = es[:, rT * 128:(rT + 1) * 128]
                nc.gpsimd.affine_select(
                    out=psel[:], in_=blk, compare_op=ALU.is_equal,
                    fill=0.0, base=0, pattern=[[-1, 128]],
                    channel_multiplier=1)
                nc.vector.tensor_reduce(pose[:, rT:rT + 1], psel[:],
                                        axis=mybir.AxisListType.X, op=ALU.add)

        mm_bufs = 2 if ncomp <= 1024 else 1
        with tc.tile_pool(name="mm_ps", bufs=mm_bufs, space="PSUM") as mmp, \
             tc.tile_pool(name="es", bufs=3) as esp, \
             tc.tile_pool(name="cacc_ps", bufs=1, space="PSUM") as cbp, \
             tc.tile_pool(name="fn", bufs=2) as fnp, \
             tc.tile_pool(name="nrm", bufs=4) as nrm, \
             tc.tile_pool(name="tp_ps", bufs=2, space="PSUM") as tps:
            cacc_ps = cbp.tile([128, 512], F32)

            def load_view(v, hb, fT):
                """Per-tile pipelined norm -> scale -> transpose."""
                for t in range(st):
                    col = v * st + t
                    sq = fnp.tile([128, D], BF16, tag="sq", name=f"sq{v}_{t}")
                    nc.scalar.activation(sq[:], hb[:, t, :], AF.Square,
                                         accum_out=ss[:, col:col + 1])
                    lncol = nrm.tile([128, 1], F32, tag="ln", name=f"ln{v}_{t}")
                    nc.scalar.activation(lncol[:], ss[:, col:col + 1], AF.Ln)
                    # 8 * rsqrt(ss) = exp(-0.5*ln(ss) + ln(8))
                    rcol = nrm.tile([128, 1], F32, tag="ri", name=f"ri{v}_{t}")
                    nc.scalar.activation(rcol[:], lncol[:], AF.Exp,
                                         scale=-0.5, bias=ln8_col[:])
                    nc.vector.tensor_mul(sc12[:, col:col + 1], rcol[:],
                                         msk[:, t:t + 1])
                    fn = fnp.tile([128, D], BF16, tag="fn", name=f"fn{v}_{t}")
                    nc.vector.tensor_scalar_mul(
                        fn[:], hb[:, t, :], sc12[:, col:col + 1])
                    pt = tps.tile([128, D], BF16, tag="pt", name=f"pt{v}_{t}")
                    for k in range(KT):
                        nc.tensor.transpose(pt[:, k * 128:(k + 1) * 128],
                                            fn[:, k * 128:(k + 1) * 128],
                                            ident[:])
                    nc.vector.tensor_copy(
                        fT[:, :, t * 128:(t + 1) * 128],
                        pt[:].rearrange("p (k c) -> p k c", k=KT))
                    if v == 0:
                        # keep the PE HAM activity monitor warm during the
                        # DMA-gated transpose phase so transposes and the
                        # first strip matmuls run at 2.4 GHz (cacc_ps is
                        # only truly used later, by the B rows, whose first
                        # matmul re-inits it with start=True)
                        nc.tensor.matmul(cacc_ps[:, :], ident[:],
                                         warm_rhs[:], start=True, stop=True,
                                         skip_group_check=True)

            load_view(0, h1b, fT1)
            # A quadrant: view-1 rows x view-1 cols; kill self-diag.
            # Emitted before view-2 prep so the PE overlaps it with the
            # view-2 DMA/norm chain.
            for r in range(st):
                row_pass(mmp, esp, r, fT1, fT1, r, (accA, r),
                         False, False, None, False, False)
            load_view(1, h2b, fT2)
            # B quadrant: view-1 rows x view-2 cols; pos diag stays in the
            # sum; accumulate column sums; extract pos
            for r in range(st):
                row_pass(mmp, esp, r, fT1, fT2, None, (accB, r),
                         True, True, cacc_ps, r == 0, r == st - 1)

            # cacc [1, ncomp] -> token layout [128, st]; overlaps D rows
            crow = esp.tile([1, ncomp], F32, tag="crow", name="crow")
            for ci, (c0, w) in enumerate(strips):
                nc.vector.tensor_copy(crow[:, c0:c0 + w],
                                      cacc_ps[32 * ci:32 * ci + 1, 0:w])
            with tc.tile_pool(name="ct_ps", bufs=1, space="PSUM") as ctp:
                ct = ctp.tile([128, st], F32)
                for c in range(st):
                    # rank-1 matmul: ct[:, c] = crow[0, 128c:128c+128]^T * 1
                    nc.tensor.matmul(ct[:, c:c + 1],
                                     crow[0:1, c * 128:(c + 1) * 128],
                                     ones_f[0:1, :], start=True, stop=True)
                nc.vector.tensor_copy(cacc[:], ct[:])

                # D quadrant: view-2 rows x view-2 cols; kill self-diag
                for r in range(st, nb):
                    row_pass(mmp, esp, r, fT2, fT2, r % st, (accB, r),
                             False, False, None, False, False)

        # ---- epilogue: per-core masked sum of ln(Ng+pos) - ln(pos) ----
        with tc.tile_pool(name="ep", bufs=1) as ep, \
             tc.tile_pool(name="ep_ps", bufs=1, space="PSUM") as epp:
            ng = ep.tile([128, nb], F32)
            nc.vector.tensor_add(ng[:, 0:st], accA[:], accB[:, 0:st])
            nc.vector.tensor_add(ng[:, st:nb], accB[:, st:nb], cacc[:])
            lg = ep.tile([128, nb], F32)
            # ln(rowsum + negK0) = ln(Ng + pos)
            nc.scalar.activation(lg[:], ng[:], AF.Ln, bias=negK0)
            plog = ep.tile([128, st], F32)
            nc.scalar.activation(plog[:], pose[:], AF.Ln)
            ptok = ep.tile([128, nb], F32)
            nc.vector.tensor_sub(ptok[:, 0:st], lg[:, 0:st], plog[:])
            nc.vector.tensor_sub(ptok[:, st:nb], lg[:, st:nb], plog[:])
            nc.vector.tensor_mul(ptok[:, 0:st], ptok[:, 0:st], msk)
            nc.vector.tensor_mul(ptok[:, st:nb], ptok[:, st:nb], msk)
            tsum = ep.tile([128, 1], F32)
            nc.vector.tensor_reduce(tsum[:], ptok[:],
                                    axis=mybir.AxisListType.X, op=ALU.add)
            lps = epp.tile([1, 1], F32)
            nc.tensor.matmul(lps[:], ones_f[:], tsum[:], start=True, stop=True)
            lsb = ep.tile([1, 1], F32)
            nc.vector.tensor_copy(lsb[:], lps[:])
            nc.sync.dma_start(out[:], lsb[:])

    return nc


_NC_CACHE: dict = {}


def _get_nc(ncomp: int) -> bass.Bass:
    if ncomp not in _NC_CACHE:
        _NC_CACHE[ncomp] = _build(ncomp)
    return _NC_CACHE[ncomp]


def _prep_core(h1_b: np.ndarray, h2_b: np.ndarray, mask_b: np.ndarray,
               ncomp: int):
    idx = np.nonzero(mask_b)[0]
    n = idx.shape[0]
    idx_pad = np.zeros(ncomp, dtype=np.int64)
    idx_pad[:n] = idx
    st = ncomp // 128
    maskc = np.zeros(ncomp, dtype=np.float32)
    maskc[:n] = 1.0
    aux = np.empty((128, st + 1), dtype=np.float32)
    aux[:, 0:st] = maskc.reshape(st, 128).T
    aux[:, st] = 2.0 * n - 2.0 * ncomp
    return ({"h1": np.ascontiguousarray(h1_b[idx_pad], dtype=np.float32),
             "h2": np.ascontiguousarray(h2_b[idx_pad], dtype=np.float32),
             "aux": aux}, n)


def _in_maps(last_hidden_states_1, last_hidden_states_2, token_mask_batch):
    h1 = np.asarray(last_hidden_states_1, dtype=np.float32)
    h2 = np.asarray(last_hidden_states_2, dtype=np.float32)
    mask = np.asarray(token_mask_batch).astype(bool)
    assert h1.shape == (NCORES, S, D), h1.shape
    max_n = int(mask.sum(axis=1).max())
    ncomp = max(NCOMP, -(-max_n // 128) * 128)
    maps, ns = [], []
    for b in range(NCORES):
        m, n = _prep_core(h1[b], h2[b], mask[b], ncomp)
        maps.append(m)
        ns.append(n)
    return maps, ns, ncomp


def kernel(last_hidden_states_1, last_hidden_states_2, token_mask_batch):
    maps, ns, ncomp = _in_maps(last_hidden_states_1, last_hidden_states_2,
                               token_mask_batch)
    nc = _get_nc(ncomp)
    res = run_bass_kernel_spmd(nc, maps, list(range(NCORES)))
    per_core = [
        float(np.asarray(res.results[b]["loss"], dtype=np.float32).reshape(()))
        / (2.0 * ns[b])
        for b in range(NCORES)
    ]
    return np.float32(np.mean(per_core))


# revision 17
# speedup vs baseline: 1.1869x; 1.1869x over previous
"""ContraCLM token-level contrastive loss on 8 Trainium2 NeuronCores.

Data-parallel over the batch: core b handles sample b (B=8). The token
mask is known on the host inside kernel(), so unmasked tokens are
COMPACTED host-side to a fixed padded length NCOMP (= 896 >= n + 6.5
sigma for n ~ Binom(1536, 1/2)); pad slots reuse token 0's row and get
mask 0. The device then works on a dense [2*NCOMP, 2*NCOMP] problem --
2.9x fewer matmul/exp elements than the full 2S grid.

Per core, with N_c = NCOMP, D = 1024, T = 0.05:

  f_v = l2norm(h_v) masked; rsqrt computed as exp(-0.5*ln(ss)) so the
  ScalarE only ever needs the natural_log_exp activation-table set.
  F = [f1; f2], stored transposed as fp8 [D, 2*N_c] (x8 scale).

  sim strips: per row-block r, one PSUM group [128, N_c] (<=2 banks)
  accumulates 4 DoubleRow fp8 matmuls per 512-col strip (K=1024).
  Self-similarity diagonals are killed by ONE extra tiny matmul that
  accumulates -30000*I into the diagonal 128-block, so exp() of the
  whole group rowsum-accumulates on the ScalarE free-dim accumulator
  with no fixups. The positive-pair column (block diagonal of the B
  quadrant) stays IN the row sum (denominator = Ng + pos), and pos
  itself is extracted from the exp'd block via affine_select(diag).

  C-quadrant row sums (view-2 rows vs view-1 cols) are B-quadrant
  column sums by symmetry: ones-weight matmuls accumulate [1, 512]
  PSUM rows which are transposed back to token layout on the PE.

  Masked/pad columns contribute exp(0)=1 to every row sum: corrected
  with negK0 = 2n - 2*N_c (host-computed, folded into the Ln bias).
  per-core output = sum_tok mask*(ln(Ng+pos) - ln(pos)); the host
  divides by 2n and averages across the 8 cores (no device collective).
"""

import sys

for _p in ("/opt/trn_rl_repo", "/opt/pypackages"):
    if _p not in sys.path:
        sys.path.append(_p)

from contextlib import ExitStack

import numpy as np

import bass_rust

import concourse.bass as bass
import concourse.tile as tile
from concourse import mybir
from concourse.bass_utils import run_bass_kernel_spmd
from concourse.masks import make_identity
from concourse.vector_clock import ScopedClock

# The walrus build in this container encodes at most 2 sync waits per
# instruction (bass_rust's inst_waits_full agrees), but Tile's semaphore
# assignment can attach more. Hoist excess waits onto unfusable same-engine
# NoOps immediately before the instruction -- the engine executes its queue
# in order, so semantics are preserved.
_MAX_WAITS = 1


def _split_excess_waits(nc, ordered):
    for bb_name, insts in ordered.items():
        out = []
        changed = False
        for inst in insts:
            si = getattr(inst, "sync_info", None)
            waits = list(si.on_wait) if si is not None else []
            if len(waits) > _MAX_WAITS:
                changed = True
                extra, keep = waits[:-_MAX_WAITS], waits[-_MAX_WAITS:]
                for i in range(0, len(extra), _MAX_WAITS):
                    out.append(mybir.InstNoOp(
                        name=nc.get_next_instruction_name(),
                        sync_info=mybir.SyncInfo(
                            on_wait=extra[i:i + _MAX_WAITS], on_update=[]),
                        bass_nofuse=True,
                        engine=inst.engine,
                    ))
                si.on_wait = keep
            out.append(inst)
        if changed:
            insts[:] = out


_orig_lower_ordered_insts = tile.TileContext._lower_ordered_insts


def _patched_lower_ordered_insts(self, ordered):
    _split_excess_waits(self.nc, ordered)
    return _orig_lower_ordered_insts(self, ordered)


tile.TileContext._lower_ordered_insts = _patched_lower_ordered_insts


def _split_waits_drain_and_barrier(self, tick_clock, wait_clock):
    nc = self.nc
    probe = nc.sync.nop(nofuse=True)
    wait_clock.add_sem_waits(
        probe.ins, ScopedClock({None: tick_clock.global_clock}))
    si = probe.ins.sync_info
    waits = list(si.on_wait) if si is not None else []
    if len(waits) > _MAX_WAITS:
        si.on_wait = waits[:_MAX_WAITS]
        for i in range(_MAX_WAITS, len(waits), _MAX_WAITS):
            nxt = nc.sync.nop(nofuse=True)
            nxt.ins.sync_info = bass_rust.SyncInfo(
                on_wait=waits[i:i + _MAX_WAITS], on_update=[])
    nc.sync.drain()
    nc.all_engine_barrier()
    assert self.sems is not None
    popped = nc._tile_sem_poison_stack.pop()
    assert popped is self._sem_poison
    nc.clear_and_free_semaphores(list(self.sems.allocated().values()))
    nc.all_engine_barrier()


tile.TileContext._drain_and_barrier = _split_waits_drain_and_barrier

S, D, NCORES = 1536, 1024, 8
NCOMP = 896              # padded compacted token count (multiple of 128)
KT = D // 128            # 8 contraction k-tiles
TEMP_INV = 20.0          # 1 / 0.05
FP8_SCALE = 8.0          # f entries ~N(0, 1/32); x8 keeps them in e4m3 range
NEGBIG = -30000.0        # diag killer: exp(scale*(64*sim + NEGBIG)) == 0
F32 = mybir.dt.float32
BF16 = mybir.dt.bfloat16
FP8 = mybir.dt.float8e4
AF = mybir.ActivationFunctionType
ALU = mybir.AluOpType


def _col_strips(sc):
    """Split sc (= NCOMP, multiple of 128) into <=512-wide col strips."""
    strips = []
    c = 0
    while c < sc:
        w = min(512, sc - c)
        strips.append((c, w))
        c += w
    return strips


def _build(ncomp: int) -> bass.Bass:
    st = ncomp // 128          # token tiles per view
    nb = 2 * st                # row blocks of F
    strips = _col_strips(ncomp)
    exp_scale = TEMP_INV / (FP8_SCALE * FP8_SCALE)

    nc = bass.Bass(num_devices=NCORES)
    h1 = nc.dram_tensor("h1", [ncomp, D], F32, kind="ExternalInput")
    h2 = nc.dram_tensor("h2", [ncomp, D], F32, kind="ExternalInput")
    # aux: cols 0..st-1 = mask in [128, st] layout (token t = 128*col+row),
    # col st = negK0 = 2n - 2*ncomp (broadcast down the partition dim)
    aux = nc.dram_tensor("aux", [128, st + 1], F32, kind="ExternalInput")
    out = nc.dram_tensor("loss", [1, 1], F32, kind="ExternalOutput")

    with tile.TileContext(nc) as tc, ExitStack() as ctx:
        const_pool = ctx.enter_context(tc.tile_pool(name="const", bufs=1))
        big = ctx.enter_context(tc.tile_pool(name="big", bufs=1))
        stat = ctx.enter_context(tc.tile_pool(name="stat", bufs=1))

        # Kick off all input DMAs first, split across two trigger queues
        # (Sync and Tensor) so descriptor generation doesn't serialize.
        h1b = big.tile([128, st, D], F32)
        h2b = big.tile([128, st, D], F32)
        auxs = const_pool.tile([128, st + 1], F32)
        nc.sync.dma_start(auxs[:], aux[:])
        for t in range(st):
            nc.sync.dma_start(h1b[:, t, :], h1[t * 128:(t + 1) * 128, :])
        for t in range(st):
            nc.sync.dma_start(h2b[:, t, :], h2[t * 128:(t + 1) * 128, :])
        msk = auxs[:, 0:st]
        negK0 = auxs[:, st:st + 1]

        ident = const_pool.tile([128, 128], BF16)
        make_identity(nc, ident[:])
        negbigI = const_pool.tile([128, 128], BF16)
        nc.vector.tensor_scalar_mul(negbigI[:], ident[:], NEGBIG)
        ones_bf = const_pool.tile([128, 1], BF16)
        nc.gpsimd.memset(ones_bf[:], 1.0)
        ones_f = const_pool.tile([128, 1], F32)
        nc.gpsimd.memset(ones_f[:], 1.0)
        ln8_col = const_pool.tile([128, 1], F32)
        nc.gpsimd.memset(ln8_col[:], float(np.log(FP8_SCALE)))
        warm_rhs = const_pool.tile([128, 512], BF16)
        nc.gpsimd.memset(warm_rhs[:], 0.0)

        fT1 = big.tile([128, KT, ncomp], FP8)     # f1^T * 8
        fT2 = big.tile([128, KT, ncomp], FP8)
        ss = stat.tile([128, nb], F32)            # |h|^2 per token, both views
        sc12 = stat.tile([128, nb], F32)          # 8 * mask * rsqrt(ss)
        accA = stat.tile([128, st], F32)          # A-quadrant row sums
        accB = stat.tile([128, nb], F32)          # B+D quadrant row sums
        pose = stat.tile([128, st], F32)          # exp(pos_sim/T) extracted
        cacc = stat.tile([128, st], F32)          # B col sums (C row sums)

        # ---- sim strips: 3 half-rows of row passes ----
        def row_pass(mmp, esp, r, lhsT, rhsT, dk_block, acc_col, want_cacc,
                     want_pos, cacc_ps, first_cacc, last_cacc):
            rT = r % st
            ps = mmp.tile([128, 1024 * ((ncomp + 1023) // 1024)], F32,
                          tag="ps", name=f"ps_{r}_{acc_col[1]}")
            for g in range(KT // 2):
                for (c0, w) in strips:
                    nc.tensor.matmul(
                        ps[:, c0:c0 + w],
                        lhsT[:, 2 * g:2 * g + 2, rT * 128:(rT + 1) * 128],
                        rhsT[:, 2 * g:2 * g + 2, c0:c0 + w],
                        perf_mode=mybir.MatmulPerfMode.DoubleRow,
                        start=(g == 0),
                        stop=(g == KT // 2 - 1 and dk_block is None))
            if dk_block is not None:
                b0 = dk_block * 128
                nc.tensor.matmul(ps[:, b0:b0 + 128], negbigI[:], ident[:],
                                 start=False, stop=True, skip_group_check=True)
            es = esp.tile([128, ncomp], BF16, tag="es", name=f"es_{r}_{acc_col[1]}")
            nc.scalar.activation(es[:], ps[:, 0:ncomp], AF.Exp,
                                 scale=exp_scale,
                                 accum_out=acc_col[0][:, acc_col[1]:acc_col[1] + 1])
            if want_cacc:
                for ci, (c0, w) in enumerate(strips):
                    nc.tensor.matmul(
                        cacc_ps[32 * ci:32 * ci + 1, 0:w],
                        ones_bf[:], es[:, c0:c0 + w],
                        start=first_cacc, stop=last_cacc,
                        skip_group_check=True)
            if want_pos:
                psel = esp.tile([128, 128], BF16, tag="psel", name=f"psel_{r}")
                blk = es[:, rT * 128:(rT + 1) * 128]
                nc.gpsimd.affine_select(
                    out=psel[:], in_=blk, compare_op=ALU.is_equal,
                    fill=0.0, base=0, pattern=[[-1, 128]],
                    channel_multiplier=1)
                nc.vector.tensor_reduce(pose[:, rT:rT + 1], psel[:],
                                        axis=mybir.AxisListType.X, op=ALU.add)

        mm_bufs = 2 if ncomp <= 1024 else 1
        with tc.tile_pool(name="mm_ps", bufs=mm_bufs, space="PSUM") as mmp, \
             tc.tile_pool(name="es", bufs=3) as esp, \
             tc.tile_pool(name="cacc_ps", bufs=1, space="PSUM") as cbp, \
             tc.tile_pool(name="fn", bufs=2) as fnp, \
             tc.tile_pool(name="nrm", bufs=4) as nrm, \
             tc.tile_pool(name="tp_ps", bufs=2, space="PSUM") as tps:
            cacc_ps = cbp.tile([128, 512], F32)

            def tile_chain(v, t, hb, fT):
                """Per-tile pipelined norm -> scale -> transpose."""
                col = v * st + t
                sq = fnp.tile([128, D], BF16, tag="sq", name=f"sq{v}_{t}")
                nc.scalar.activation(sq[:], hb[:, t, :], AF.Square,
                                     accum_out=ss[:, col:col + 1])
                lncol = nrm.tile([128, 1], F32, tag="ln", name=f"ln{v}_{t}")
                nc.scalar.activation(lncol[:], ss[:, col:col + 1], AF.Ln)
                # 8 * rsqrt(ss) = exp(-0.5*ln(ss) + ln(8))
                rcol = nrm.tile([128, 1], F32, tag="ri", name=f"ri{v}_{t}")
                nc.scalar.activation(rcol[:], lncol[:], AF.Exp,
                                     scale=-0.5, bias=ln8_col[:])
                nc.vector.tensor_mul(sc12[:, col:col + 1], rcol[:],
                                     msk[:, t:t + 1])
                fn = fnp.tile([128, D], BF16, tag="fn", name=f"fn{v}_{t}")
                nc.vector.tensor_scalar_mul(
                    fn[:], hb[:, t, :], sc12[:, col:col + 1])
                pt = tps.tile([128, D], BF16, tag="pt", name=f"pt{v}_{t}")
                for k in range(KT):
                    nc.tensor.transpose(pt[:, k * 128:(k + 1) * 128],
                                        fn[:, k * 128:(k + 1) * 128],
                                        ident[:])
                nc.vector.tensor_copy(
                    fT[:, :, t * 128:(t + 1) * 128],
                    pt[:].rearrange("p (k c) -> p k c", k=KT))
                if v == 0:
                    # keep the PE HAM activity monitor warm during the
                    # DMA-gated transpose phase so transposes and the
                    # first strip matmuls run at 2.4 GHz (cacc_ps is
                    # only truly used later, by the B rows, whose first
                    # matmul re-inits it with start=True)
                    nc.tensor.matmul(cacc_ps[:, :], ident[:],
                                     warm_rhs[:], start=True, stop=True,
                                     skip_group_check=True)

            for t in range(st):
                tile_chain(0, t, h1b, fT1)
            # A-quadrant rows interleaved with view-2 tile prep: the A
            # matmuls/exps fill the PE/ScalarE gaps while view-2 DMAs land.
            for i in range(st):
                row_pass(mmp, esp, i, fT1, fT1, i, (accA, i),
                         False, False, None, False, False)
                tile_chain(1, i, h2b, fT2)
            # B quadrant: view-1 rows x view-2 cols; pos diag stays in the
            # sum; accumulate column sums; extract pos
            for r in range(st):
                row_pass(mmp, esp, r, fT1, fT2, None, (accB, r),
                         True, True, cacc_ps, r == 0, r == st - 1)

            # cacc [1, ncomp] -> token layout [128, st]; overlaps D rows
            crow = esp.tile([1, ncomp], F32, tag="crow", name="crow")
            for ci, (c0, w) in enumerate(strips):
                nc.vector.tensor_copy(crow[:, c0:c0 + w],
                                      cacc_ps[32 * ci:32 * ci + 1, 0:w])
            with tc.tile_pool(name="ct_ps", bufs=1, space="PSUM") as ctp:
                ct = ctp.tile([128, st], F32)
                for c in range(st):
                    # rank-1 matmul: ct[:, c] = crow[0, 128c:128c+128]^T * 1
                    nc.tensor.matmul(ct[:, c:c + 1],
                                     crow[0:1, c * 128:(c + 1) * 128],
                                     ones_f[0:1, :], start=True, stop=True)
                nc.vector.tensor_copy(cacc[:], ct[:])

                # D quadrant: view-2 rows x view-2 cols; kill self-diag
                for r in range(st, nb):
                    row_pass(mmp, esp, r, fT2, fT2, r % st, (accB, r),
                             False, False, None, False, False)

        # ---- epilogue: per-core masked sum of ln(Ng+pos) - ln(pos) ----
        with tc.tile_pool(name="ep", bufs=1) as ep, \
             tc.tile_pool(name="ep_ps", bufs=1, space="PSUM") as epp:
            ng = ep.tile([128, nb], F32)
            nc.vector.tensor_add(ng[:, 0:st], accA[:], accB[:, 0:st])
            nc.vector.tensor_add(ng[:, st:nb], accB[:, st:nb], cacc[:])
            lg = ep.tile([128, nb], F32)
            # ln(rowsum + negK0) = ln(Ng + pos)
            nc.scalar.activation(lg[:], ng[:], AF.Ln, bias=negK0)
            plog = ep.tile([128, st], F32)
            nc.scalar.activation(plog[:], pose[:], AF.Ln)
            ptok = ep.tile([128, nb], F32)
            nc.vector.tensor_sub(ptok[:, 0:st], lg[:, 0:st], plog[:])
            nc.vector.tensor_sub(ptok[:, st:nb], lg[:, st:nb], plog[:])
            nc.vector.tensor_mul(ptok[:, 0:st], ptok[:, 0:st], msk)
            nc.vector.tensor_mul(ptok[:, st:nb], ptok[:, st:nb], msk)
            tsum = ep.tile([128, 1], F32)
            nc.vector.tensor_reduce(tsum[:], ptok[:],
                                    axis=mybir.AxisListType.X, op=ALU.add)
            lps = epp.tile([1, 1], F32)
            nc.tensor.matmul(lps[:], ones_f[:], tsum[:], start=True, stop=True)
            lsb = ep.tile([1, 1], F32)
            nc.vector.tensor_copy(lsb[:], lps[:])
            nc.sync.dma_start(out[:], lsb[:])

    return nc


_NC_CACHE: dict = {}


def _get_nc(ncomp: int) -> bass.Bass:
    if ncomp not in _NC_CACHE:
        _NC_CACHE[ncomp] = _build(ncomp)
    return _NC_CACHE[ncomp]


def _prep_core(h1_b: np.ndarray, h2_b: np.ndarray, mask_b: np.ndarray,
               ncomp: int):
    idx = np.nonzero(mask_b)[0]
    n = idx.shape[0]
    idx_pad = np.zeros(ncomp, dtype=np.int64)
    idx_pad[:n] = idx
    st = ncomp // 128
    maskc = np.zeros(ncomp, dtype=np.float32)
    maskc[:n] = 1.0
    aux = np.empty((128, st + 1), dtype=np.float32)
    aux[:, 0:st] = maskc.reshape(st, 128).T
    aux[:, st] = 2.0 * n - 2.0 * ncomp
    return ({"h1": np.ascontiguousarray(h1_b[idx_pad], dtype=np.float32),
             "h2": np.ascontiguousarray(h2_b[idx_pad], dtype=np.float32),
             "aux": aux}, n)


def _in_maps(last_hidden_states_1, last_hidden_states_2, token_mask_batch):
    h1 = np.asarray(last_hidden_states_1, dtype=np.float32)
    h2 = np.asarray(last_hidden_states_2, dtype=np.float32)
    mask = np.asarray(token_mask_batch).astype(bool)
    assert h1.shape == (NCORES, S, D), h1.shape
    max_n = int(mask.sum(axis=1).max())
    ncomp = max(NCOMP, -(-max_n // 128) * 128)
    maps, ns = [], []
    for b in range(NCORES):
        m, n = _prep_core(h1[b], h2[b], mask[b], ncomp)
        maps.append(m)
        ns.append(n)
    return maps, ns, ncomp


def kernel(last_hidden_states_1, last_hidden_states_2, token_mask_batch):
    maps, ns, ncomp = _in_maps(last_hidden_states_1, last_hidden_states_2,
                               token_mask_batch)
    nc = _get_nc(ncomp)
    res = run_bass_kernel_spmd(nc, maps, list(range(NCORES)))
    per_core = [
        float(np.asarray(res.results[b]["loss"], dtype=np.float32).reshape(()))
        / (2.0 * ns[b])
        for b in range(NCORES)
    ]
    return np.float32(np.mean(per_core))
